# revision 1
# baseline (speedup 1.0000x reference)
"""Trainium2 Bass kernel for nn_CustomConv2d_32538672234916.

out[b,o,h,w] = K - sum_{ci,kh,kw} exp(x_patch)*exp(w) + bias[o],  K = Cin*kh*kw = 576
i.e. out = (K + bias) - conv2d(exp(x) [1-padded], exp(weight), stride 1)

Sharding: data-parallel over batch B=16 across 8 cores (2 batches/core),
weights/bias replicated.

Per-core GEMM formulation: for each 8-row output strip (512 pixels), the
3x3-tap conv is 6 accumulating matmuls into one PSUM tile [Cout=128, 512]:
  - 3 matmuls with K=128: taps (kh=0,kw) and (kh=1,kw) packed along the
    contraction dim. SBUF holds exp(x) twice: partitions 0-63 = padded
    exp(x), partitions 64-127 = same shifted down one image row, so one
    access pattern feeds both taps.
  - 3 matmuls with K=64: taps (kh=2,kw) read from the unshifted half.
Matmul operands are bf16 (fp32 PSUM accumulation); the weight-stationary
taps-outer/strips-inner order keeps LDWEIGHTS off the critical path.
Measured steady state ~16-18us/core vs ~17.5us HBM-roofline (6.3MB/core
mandatory traffic at 360GB/s) -- the 'ridge' regime target.
"""
import sys
sys.path.insert(0, '/opt/trn_rl_repo')
import numpy as np

B, CIN, H, W = 16, 64, 64, 64
COUT = 128
NCORES = 8
BL = B // NCORES          # batches per core
PAD_W = W + 2             # 66
PAD_TOT = PAD_W * (H + 2) # 66*66 = 4356
KSUM = float(CIN * 9)     # 576
ROWS_PER_TILE = 8
NTILES = H // ROWS_PER_TILE  # 8 strips per image

_CACHE = {}


def _build(reps=1, order="strip", mm_dtype="f32r", diag="full", xin="narrow"):
    from concourse import bacc, mybir
    from concourse.tile import TileContext

    f32 = mybir.dt.float32
    mmdt = {"f32r": mybir.dt.float32r, "bf16": mybir.dt.bfloat16}[mm_dtype]
    Exp = mybir.ActivationFunctionType.Exp

    nc = bacc.Bacc("TRN2", target_bir_lowering=False, debug=False)
    x_d = nc.dram_tensor("x", [BL, CIN, H, W], f32, kind="ExternalInput")
    wpair_d = nc.dram_tensor("wpair", [128, 3 * COUT], mmdt, kind="ExternalInput")
    wsing_d = nc.dram_tensor("wsing", [64, 3 * COUT], mmdt, kind="ExternalInput")
    bvec_d = nc.dram_tensor("bvec", [COUT, 1], f32, kind="ExternalInput")
    out_d = nc.dram_tensor("out", [BL, COUT, H, W], f32, kind="ExternalOutput")
    x_ap = x_d.ap()
    out_ap = out_d.ap()

    npsum = 4 if order == "strip" else 1
    nstage = 3 if diag == "deep" else 2
    nres = 8 if diag == "deep" else 4

    with TileContext(nc) as tc:
        with tc.tile_pool(name="consts", bufs=1) as consts, \
             tc.tile_pool(name="xp", bufs=nstage) as xp, \
             tc.tile_pool(name="ep", bufs=nstage) as ep, \
             tc.tile_pool(name="rp", bufs=nres) as rp, \
             tc.tile_pool(name="pp", bufs=npsum, space="PSUM") as pp:
            wpair_t = consts.tile([128, 3 * COUT], mmdt)
            wsing_t = consts.tile([64, 3 * COUT], mmdt)
            bv_t = consts.tile([COUT, 1], f32)

            def load_consts():
                nc.sync.dma_start(wpair_t[:], wpair_d.ap())
                nc.sync.dma_start(wsing_t[:], wsing_d.ap())
                nc.sync.dma_start(bv_t[:], bvec_d.ap())

            # out-DMA granularity: strips per transfer (bigger = fewer,
            # more efficient DMAs; epilogues within a group share one tile)
            OUTG = {"pair2": 2, "pair4": 4}.get(diag, 1)
            SPB = ROWS_PER_TILE * W  # elements per strip
            epi_state = {}

            def epilogue(b, t, pt):
                slot = t % OUTG
                if slot == 0:
                    epi_state["res"] = rp.tile([COUT, OUTG * SPB], f32,
                                               tag="res", name=f"res_{b}_{t}")
                res = epi_state["res"]
                nc.vector.tensor_scalar(res[:, slot * SPB:(slot + 1) * SPB],
                                        pt[:], -1.0, bv_t[:],
                                        mybir.AluOpType.mult,
                                        mybir.AluOpType.add)
                if slot == OUTG - 1:
                    t0 = t - (OUTG - 1)
                    nc.sync.dma_start(
                        out_ap[b][:, t0 * ROWS_PER_TILE:(t + 1) * ROWS_PER_TILE, :],
                        res[:])

            ets = {}
            HH = H // 2
            for i, b in enumerate([b for _ in range(reps) for b in range(BL)]):
                if xin == "wide":
                    # x spread over all 128 partitions (full 16 DMA ports):
                    # partition ci      <- x[b, ci, 0:32, :]
                    # partition 64+ci   <- x[b, ci, 32:64, :]
                    xt = xp.tile([128, HH * W], f32, tag="xt", name=f"xt_{i}")
                    nc.sync.dma_start(
                        xt[:], x_ap[b].rearrange("c (s h) w -> s c (h w)", s=2))
                else:
                    xt = xp.tile([CIN, H * W], f32, tag="xt", name=f"xt_{i}")
                    in_dma = (nc.gpsimd.dma_start if diag == "swin"
                              else nc.sync.dma_start)
                    in_dma(xt[:], x_ap[b].rearrange("c h w -> c (h w)"))
                if i == 0:
                    # after the first x DMA so x-data flows immediately
                    load_consts()
                xt3 = xt.rearrange("p (h w) -> p h w", w=W)
                bufslot = i % nstage
                if bufslot not in ets:
                    # exp(pad)=1.0 cells are written once per buffer and
                    # persist (later iterations only rewrite the interiors)
                    et = ep.tile([128, PAD_TOT], mmdt, tag="et",
                                 name=f"et_{bufslot}")
                    ets[bufslot] = et
                    e3 = et.rearrange("p (h w) -> p h w", w=PAD_W)
                    nc.vector.memset(e3[0:64, 0, :], 1.0)          # top pad row
                    nc.vector.memset(e3[0:64, H + 1, :], 1.0)      # bottom pad row
                    nc.vector.memset(e3[0:64, 1:H + 1, 0], 1.0)    # left pad col
                    nc.vector.memset(e3[0:64, 1:H + 1, W + 1], 1.0)  # right pad col
                    if xin == "wide":
                        # half1 rows 32..63 pads are never rewritten either
                        nc.vector.memset(e3[64:128, HH:H, 0], 1.0)
                        nc.vector.memset(e3[64:128, HH:H, W + 1], 1.0)
                et = ets[bufslot]
                et3 = et.rearrange("p (h w) -> p h w", w=PAD_W)
                # half0 (partitions 0-63) = padded exp(x); half1 (64-127) =
                # same shifted down one padded row (tap kh=1 reads it at the
                # kh=0 offsets).
                if xin == "wide":
                    # exp stays same-partition; the two shifted-copy DMAs move
                    # the opposite quadrants across the partition halves.
                    nc.scalar.activation(et3[0:CIN, 1:HH + 1, 1:W + 1],
                                         xt3[0:64], Exp)
                    # half1 rows 0..31  <- half0 rows 1..32 (pads included)
                    nc.sync.dma_start(et[64:128, 0:HH * PAD_W],
                                      et[0:64, PAD_W:(HH + 1) * PAD_W])
                    nc.scalar.activation(et3[64:128, HH:H, 1:W + 1],
                                         xt3[64:128], Exp)
                    # half0 rows 33..64 <- half1 rows 32..63 (pads included)
                    nc.sync.dma_start(
                        et[0:64, (HH + 1) * PAD_W:(H + 1) * PAD_W],
                        et[64:128, HH * PAD_W:H * PAD_W])
                else:
                    dup_dma = (nc.gpsimd.dma_start if diag == "swin"
                               else nc.sync.dma_start)
                    nc.scalar.activation(et3[0:CIN, 1:HH + 1, 1:W + 1],
                                         xt3[:, 0:HH, :], Exp)
                    if diag != "nodup":
                        dup_dma(et[64:128, 0:HH * PAD_W],
                                et[0:64, PAD_W:(HH + 1) * PAD_W])
                    nc.scalar.activation(et3[0:CIN, HH + 1:H + 1, 1:W + 1],
                                         xt3[:, HH:H, :], Exp)
                    if diag != "nodup":
                        dup_dma(et[64:128, HH * PAD_W:PAD_TOT - PAD_W],
                                et[0:64, (HH + 1) * PAD_W:PAD_TOT])
                    elif (i, bufslot) in ((0, 0), (1, 1)):
                        nc.vector.memset(et[64:128, :], 1.0)

                def rhs_pair(t, dx):
                    h0 = t * ROWS_PER_TILE
                    return et3[0:128, h0:h0 + ROWS_PER_TILE, dx:dx + W]

                def rhs_sing(t, dx):
                    h0 = t * ROWS_PER_TILE
                    return et3[0:64, h0 + 2:h0 + 2 + ROWS_PER_TILE, dx:dx + W]

                if order == "strip":
                    for t in range(NTILES):
                        pt = pp.tile([COUT, ROWS_PER_TILE * W], f32, tag="pt",
                                     name=f"pt_{i}_{t}")
                        for dx in range(3):
                            nc.tensor.matmul(
                                pt[:], wpair_t[:, dx * COUT:(dx + 1) * COUT],
                                rhs_pair(t, dx), start=(dx == 0), stop=False)
                        for dx in range(3):
                            nc.tensor.matmul(
                                pt[:], wsing_t[:, dx * COUT:(dx + 1) * COUT],
                                rhs_sing(t, dx), start=False, stop=(dx == 2))
                        epilogue(b, t, pt)
                else:  # taps outer within groups of G strips
                    G = {"tap": NTILES, "tapb": NTILES, "tap4": 4, "tap2": 2}[order]
                    ntag, nbuf = (4, 2) if order == "tapb" else (8, 1)
                    nsing = 0 if diag == "halfmm" else 3
                    for g0 in range(0, NTILES, G):
                        strips = range(g0, g0 + G)
                        pts = {t: pp.tile([COUT, ROWS_PER_TILE * W], f32,
                                          tag=f"pt{t % ntag}", bufs=nbuf,
                                          name=f"pt_{i}_{t}")
                               for t in strips}
                        for dx in range(3):
                            for t in strips:
                                nc.tensor.matmul(
                                    pts[t][:],
                                    wpair_t[:, dx * COUT:(dx + 1) * COUT],
                                    rhs_pair(t, dx), start=(dx == 0),
                                    stop=(dx == 2 and nsing == 0))
                                if dx == 2 and nsing == 0:
                                    epilogue(b, t, pts[t])
                        for dx in range(nsing):
                            for t in strips:
                                nc.tensor.matmul(
                                    pts[t][:],
                                    wsing_t[:, dx * COUT:(dx + 1) * COUT],
                                    rhs_sing(t, dx), start=False, stop=(dx == 2))
                                if dx == 2:
                                    epilogue(b, t, pts[t])
    nc.compile()
    return nc


def _prep_weights(weight, bias, mm_dtype="f32r"):
    # wpair[ci, dx*128+o] = exp(w[o,ci,0,dx]); wpair[64+ci, ...] = exp(w[o,ci,1,dx])
    ew = np.exp(weight.astype(np.float32))           # [COUT, CIN, 3, 3]
    wpair = np.empty((128, 3 * COUT), np.float32)
    wsing = np.empty((64, 3 * COUT), np.float32)
    for dx in range(3):
        wpair[0:64, dx * COUT:(dx + 1) * COUT] = ew[:, :, 0, dx].T
        wpair[64:128, dx * COUT:(dx + 1) * COUT] = ew[:, :, 1, dx].T
        wsing[:, dx * COUT:(dx + 1) * COUT] = ew[:, :, 2, dx].T
    if mm_dtype == "bf16":
        import ml_dtypes
        wpair = wpair.astype(ml_dtypes.bfloat16)
        wsing = wsing.astype(ml_dtypes.bfloat16)
    bvec = (KSUM + bias.astype(np.float32)).reshape(COUT, 1)
    return wpair, wsing, bvec


ORDER = "tap"
MM_DTYPE = "bf16"


def kernel(x, weight, bias):
    from concourse import bass_utils

    x = np.ascontiguousarray(np.asarray(x, dtype=np.float32))
    weight = np.asarray(weight, dtype=np.float32)
    bias = np.asarray(bias, dtype=np.float32)

    if "nc" not in _CACHE:
        _CACHE["nc"] = _build(order=ORDER, mm_dtype=MM_DTYPE)
    nc = _CACHE["nc"]

    wpair, wsing, bvec = _prep_weights(weight, bias, MM_DTYPE)
    in_maps = [
        {"x": x[c * BL:(c + 1) * BL], "wpair": wpair, "wsing": wsing, "bvec": bvec}
        for c in range(NCORES)
    ]
    res = bass_utils.run_bass_kernel_spmd(nc, in_maps, core_ids=list(range(NCORES)))
    return np.concatenate([r["out"] for r in res.results], axis=0)



# revision 8
# speedup vs baseline: 1.1851x; 1.1851x over previous
"""Trainium2 Bass kernel for nn_CustomConv2d_32538672234916.

out[b,o,h,w] = K - sum_{ci,kh,kw} exp(x_patch)*exp(w) + bias[o],  K = Cin*kh*kw = 576
i.e. out = (K + bias) - conv2d(exp(x) [1-padded], exp(weight), stride 1)

Sharding: data-parallel over batch B=16 across 8 cores (2 batches/core),
weights/bias replicated.

Per-core GEMM formulation (fp8 DoubleRow): SBUF holds exp(x) in fp8 three
times: region0 half0 (partitions 0-63) = padded exp(x) "E", region0 half1
(partitions 64-127) = E shifted down one padded row, region1 (free offset
+PAD_TOT) half0 = E shifted down two rows (region1 half1 is only ever
multiplied by phantom zero weights). Each 8-row output strip (512 px)
accumulates THREE MatmulPerfMode.DoubleRow fp8 matmuls (0.5 PE-cycles/row;
k-tile-dim stride = PAD_TOT, nested/non-overlapping -- the hw AP walker
rejects overlapping k-tile strides). Matmul g=kw covers the tap column:
  ktile0 -> taps (0,kw) [half0] + (1,kw) [half1]
  ktile1 -> taps (2,kw) [region1 half0] + phantom w=0 [region1 half1]
Tensor time ~= 16 strips * 3 * 256 cyc = 5.1us/core.

IO: x uploaded bf16 (1MB/core), exp runs bf16->fp8 on the Act engine, out
written bf16 (2MB/core) and widened to f32 on host. Epilogue (PSUM f32 *-1
+ (K+bias) -> bf16) alternates DVE / GpSimd so neither engine is the wall.
fp8 end-to-end rel err ~7e-3 vs the 2e-2 gate.
"""
import sys
sys.path.insert(0, '/opt/trn_rl_repo')
import numpy as np

B, CIN, H, W = 16, 64, 64, 64
COUT = 128
NCORES = 8
BL = B // NCORES          # batches per core
PAD_W = W + 2             # 66
PAD_TOT = PAD_W * (H + 2) # 66*66 = 4356
KSUM = float(CIN * 9)     # 576
ROWS_PER_TILE = 8
NTILES = H // ROWS_PER_TILE  # 8 strips per image
HH = H // 2

# taps covered: [(half0 tap, half1 tap), ...] per (group=kw, ktile);
# ktile0 reads region0 = (E, E+1row), ktile1 reads region1 = (E+2rows, any)
# at k-tile-dim stride PAD_TOT; None = phantom zero weight.
GROUP_TAPS = [
    [((0, 0), (1, 0)), ((2, 0), None)],
    [((0, 1), (1, 1)), ((2, 1), None)],
    [((0, 2), (1, 2)), ((2, 2), None)],
]

_CACHE = {}


def _build(reps=1):
    from concourse import bacc, mybir
    from concourse.tile import TileContext

    f32 = mybir.dt.float32
    bf16 = mybir.dt.bfloat16
    f8 = mybir.dt.float8e4
    Exp = mybir.ActivationFunctionType.Exp
    DR = mybir.MatmulPerfMode.DoubleRow

    nc = bacc.Bacc("TRN2", target_bir_lowering=False, debug=False)
    x_d = nc.dram_tensor("x", [BL, CIN, H, W], bf16, kind="ExternalInput")
    wdr_d = nc.dram_tensor("wdr", [128, 3 * 2 * COUT], f8, kind="ExternalInput")
    bvec_d = nc.dram_tensor("bvec", [COUT, 1], f32, kind="ExternalInput")
    out_d = nc.dram_tensor("out", [BL, COUT, H, W], bf16, kind="ExternalOutput")
    x_ap = x_d.ap()
    out_ap = out_d.ap()

    with TileContext(nc) as tc:
        with tc.tile_pool(name="consts", bufs=1) as consts, \
             tc.tile_pool(name="xp", bufs=2) as xp, \
             tc.tile_pool(name="ep", bufs=2) as ep, \
             tc.tile_pool(name="rp", bufs=4) as rp, \
             tc.tile_pool(name="pp", bufs=1, space="PSUM") as pp:
            wdr_t = consts.tile([128, 3 * 2 * COUT], f8)
            bv_t = consts.tile([COUT, 1], f32)
            wdr4 = wdr_t.rearrange("p (g k m) -> p g k m", g=3, k=2)

            def load_consts():
                nc.sync.dma_start(wdr_t[:], wdr_d.ap())
                nc.sync.dma_start(bv_t[:], bvec_d.ap())

            def epilogue(i, b, t, pt):
                # PSUM f32 * -1 + (K+bias) -> bf16. GPSIMD can't read PSUM on
                # this target, so split strips between DVE (13/16) and the Act
                # engine (3/16; Identity shares the loaded table with Exp).
                res = rp.tile([COUT, ROWS_PER_TILE * W], bf16, tag="res",
                              name=f"res_{b}_{t}")
                on_act = (t == 3) or (t == 6 and i % 2 == 0)
                if on_act:
                    nc.scalar.activation(res[:], pt[:],
                                         mybir.ActivationFunctionType.Identity,
                                         bias=bv_t[:], scale=-1.0)
                else:
                    nc.vector.tensor_scalar(res[:], pt[:], -1.0, bv_t[:],
                                            mybir.AluOpType.mult,
                                            mybir.AluOpType.add)
                nc.sync.dma_start(
                    out_ap[b][:, t * ROWS_PER_TILE:(t + 1) * ROWS_PER_TILE, :],
                    res[:])

            def make_rhs(et, t, kw):
                # [p, 2(k-tile, stride PAD_TOT), 8 rows, 64 cols] — nested
                # non-overlapping strides (hw rejects overlapping k-tile APs)
                e4 = et.rearrange("p (r h w) -> p r h w", r=2, w=PAD_W)
                h0 = t * ROWS_PER_TILE
                return e4[0:128, :, h0:h0 + ROWS_PER_TILE, kw:kw + W]

            ets = {}
            for i, b in enumerate([b for _ in range(reps) for b in range(BL)]):
                xt = xp.tile([CIN, H * W], bf16, tag="xt", name=f"xt_{i}")
                nc.sync.dma_start(xt[:], x_ap[b].rearrange("c h w -> c (h w)"))
                if i == 0:
                    # after the first x DMA so x-data flows immediately
                    load_consts()
                xt3 = xt.rearrange("p (h w) -> p h w", w=W)
                bufslot = i % 2
                if bufslot not in ets:
                    # pad cells are written once per buffer and persist
                    # (later iterations only rewrite the interiors)
                    et = ep.tile([128, 2 * PAD_TOT], f8, tag="et",
                                 name=f"et_{bufslot}")
                    ets[bufslot] = et
                    e3 = et.rearrange("p (h w) -> p h w", w=PAD_W)
                    nc.gpsimd.memset(e3[0:64, 0, :], 1.0)            # top pad row
                    nc.gpsimd.memset(e3[0:64, H + 1, :], 1.0)        # bottom pad
                    nc.gpsimd.memset(e3[0:64, 1:H + 1, 0], 1.0)      # left col
                    nc.gpsimd.memset(e3[0:64, 1:H + 1, W + 1], 1.0)  # right col
                    # half1 row 65 + all of region1 half1 are read only
                    # against phantom zero weights: just keep them finite
                    nc.gpsimd.memset(e3[64:128, H + 1, :], 1.0)
                    nc.gpsimd.memset(et[64:128, PAD_TOT:PAD_TOT + H * PAD_W],
                                     1.0)
                et = ets[bufslot]
                et3 = et.rearrange("p (h w) -> p h w", w=PAD_W)
                # region0: half0 (partitions 0-63) = padded exp(x); half1 =
                # same shifted down one padded row. region1 (free offset
                # +PAD_TOT): half0 = E shifted down two rows.
                nc.scalar.activation(et3[0:CIN, 1:HH + 1, 1:W + 1],
                                     xt3[:, 0:HH, :], Exp)
                nc.sync.dma_start(et[64:128, 0:HH * PAD_W],
                                  et[0:64, PAD_W:(HH + 1) * PAD_W])
                nc.scalar.activation(et3[0:CIN, HH + 1:H + 1, 1:W + 1],
                                     xt3[:, HH:H, :], Exp)
                nc.sync.dma_start(et[64:128, HH * PAD_W:PAD_TOT - PAD_W],
                                  et[0:64, (HH + 1) * PAD_W:PAD_TOT])
                nc.sync.dma_start(et[0:64, PAD_TOT:PAD_TOT + H * PAD_W],
                                  et[0:64, 2 * PAD_W:PAD_TOT])

                pts = {t: pp.tile([COUT, ROWS_PER_TILE * W], f32,
                                  tag=f"pt{t}", bufs=1, name=f"pt_{i}_{t}")
                       for t in range(NTILES)}
                for g in range(3):
                    for t in range(NTILES):
                        nc.tensor.matmul(
                            pts[t][:], wdr4[:, g, :, :], make_rhs(et, t, g),
                            start=(g == 0), stop=(g == 2), perf_mode=DR)
                        if g == 2:
                            epilogue(i, b, t, pts[t])
    nc.compile()
    return nc


def _prep_weights(weight, bias):
    """wdr[p, g, k, o]: DoubleRow stationary layout, 3 groups x 2 k-tiles.
    partition p<64 -> half0 tap weight exp(w[o, p, tap0]); p>=64 -> half1
    tap (or 0 for phantom k-tiles)."""
    import ml_dtypes
    ew = np.exp(weight.astype(np.float32))           # [COUT, CIN, 3, 3]
    wdr = np.zeros((128, 3, 2, COUT), np.float32)
    for g in range(3):
        for k in range(2):
            tap0, tap1 = GROUP_TAPS[g][k]
            wdr[0:64, g, k, :] = ew[:, :, tap0[0], tap0[1]].T
            if tap1 is not None:
                wdr[64:128, g, k, :] = ew[:, :, tap1[0], tap1[1]].T
    wdr = wdr.reshape(128, 3 * 2 * COUT).astype(ml_dtypes.float8_e4m3)
    bvec = (KSUM + bias.astype(np.float32)).reshape(COUT, 1)
    return wdr, bvec


def _in_map(x_core, weight, bias):
    """Build the per-core input map. x_core: [BL, CIN, H, W] float32."""
    import ml_dtypes
    wdr, bvec = _prep_weights(weight, bias)
    x_bf = np.ascontiguousarray(x_core.astype(ml_dtypes.bfloat16))
    return {"x": x_bf, "wdr": wdr, "bvec": bvec}


def kernel(x, weight, bias):
    from concourse import bass_utils

    x = np.ascontiguousarray(np.asarray(x, dtype=np.float32))
    weight = np.asarray(weight, dtype=np.float32)
    bias = np.asarray(bias, dtype=np.float32)

    if "nc" not in _CACHE:
        _CACHE["nc"] = _build()
    nc = _CACHE["nc"]

    in_maps = [_in_map(x[c * BL:(c + 1) * BL], weight, bias)
               for c in range(NCORES)]
    res = bass_utils.run_bass_kernel_spmd(nc, in_maps, core_ids=list(range(NCORES)))
    return np.concatenate([np.asarray(r["out"], dtype=np.float32)
                           for r in res.results], axis=0)


# revision 10
# speedup vs baseline: 1.4172x; 1.1958x over previous
"""Trainium2 Bass kernel for nn_CustomConv2d_32538672234916.

out[b,o,h,w] = K - sum_{ci,kh,kw} exp(x_patch)*exp(w) + bias[o],  K = Cin*kh*kw = 576
i.e. out = (K + bias) - conv2d(exp(x) [1-padded], exp(weight), stride 1)

Sharding: data-parallel over batch B=16 across 8 cores (2 batches/core),
weights/bias replicated.

Per-core GEMM formulation (fp8 DoubleRow): SBUF holds exp(x) in fp8 three
times: region0 half0 (partitions 0-63) = padded exp(x) "E", region0 half1
(partitions 64-127) = E shifted down one padded row, region1 (free offset
+PAD_TOT) half0 = E shifted down two rows (region1 half1 is only ever
multiplied by phantom zero weights). Each 8-row output strip (512 px)
accumulates THREE MatmulPerfMode.DoubleRow fp8 matmuls (0.5 PE-cycles/row;
k-tile-dim stride = PAD_TOT, nested/non-overlapping -- the hw AP walker
rejects overlapping k-tile strides). Matmul g=kw covers the tap column:
  ktile0 -> taps (0,kw) [half0] + (1,kw) [half1]
  ktile1 -> taps (2,kw) [region1 half0] + phantom w=0 [region1 half1]
Tensor time ~= 16 strips * 3 * 256 cyc = 5.1us/core.

IO: x uploaded bf16 (1MB/core), exp runs bf16->fp8 on the Act engine, out
written bf16 (2MB/core) and widened to f32 on host. Epilogue (PSUM f32 *-1
+ (K+bias) -> bf16) alternates DVE / GpSimd so neither engine is the wall.
fp8 end-to-end rel err ~7e-3 vs the 2e-2 gate.
"""
import sys
sys.path.insert(0, '/opt/trn_rl_repo')
import numpy as np

B, CIN, H, W = 16, 64, 64, 64
COUT = 128
NCORES = 8
BL = B // NCORES          # batches per core
PAD_W = W + 2             # 66
PAD_TOT = PAD_W * (H + 2) # 66*66 = 4356
KSUM = float(CIN * 9)     # 576
ROWS_PER_TILE = 8
NTILES = H // ROWS_PER_TILE  # 8 strips per image
HH = H // 2

# taps covered: [(half0 tap, half1 tap), ...] per (group=kw, ktile);
# ktile0 reads region0 = (E, E+1row), ktile1 reads region1 = (E+2rows, any)
# at k-tile-dim stride PAD_TOT; None = phantom zero weight.
GROUP_TAPS = [
    [((0, 0), (1, 0)), ((2, 0), None)],
    [((0, 1), (1, 1)), ((2, 1), None)],
    [((0, 2), (1, 2)), ((2, 2), None)],
]

_CACHE = {}


def _build(reps=1):
    from concourse import bacc, mybir
    from concourse.tile import TileContext

    f32 = mybir.dt.float32
    bf16 = mybir.dt.bfloat16
    f8 = mybir.dt.float8e4
    Exp = mybir.ActivationFunctionType.Exp
    DR = mybir.MatmulPerfMode.DoubleRow

    nc = bacc.Bacc("TRN2", target_bir_lowering=False, debug=False)
    x_d = nc.dram_tensor("x", [BL, CIN, H, W], bf16, kind="ExternalInput")
    wdr_d = nc.dram_tensor("wdr", [128, 3 * 2 * COUT], f8, kind="ExternalInput")
    bvec_d = nc.dram_tensor("bvec", [COUT, 1], f32, kind="ExternalInput")
    out_d = nc.dram_tensor("out", [BL, COUT, H, W], bf16, kind="ExternalOutput")
    x_ap = x_d.ap()
    out_ap = out_d.ap()

    with TileContext(nc) as tc:
        with tc.tile_pool(name="consts", bufs=1) as consts, \
             tc.tile_pool(name="xp", bufs=2) as xp, \
             tc.tile_pool(name="ep", bufs=2) as ep, \
             tc.tile_pool(name="rp", bufs=4) as rp, \
             tc.tile_pool(name="pp", bufs=1, space="PSUM") as pp:
            wdr_t = consts.tile([128, 3 * 2 * COUT], f8)
            bv_t = consts.tile([COUT, 1], f32)
            wdr4 = wdr_t.rearrange("p (g k m) -> p g k m", g=3, k=2)

            def load_consts():
                nc.sync.dma_start(wdr_t[:], wdr_d.ap())
                nc.sync.dma_start(bv_t[:], bvec_d.ap())

            # out-DMA granularity: OUTG strips per transfer. Fewer DMA
            # instructions = less (shared) descriptor-gen serialization.
            OUTG = 4
            SPB = ROWS_PER_TILE * W
            epi_state = {}

            def epilogue(i, b, t, pt):
                # PSUM f32 * -1 + (K+bias) -> bf16. GPSIMD can't read PSUM on
                # this target, so split strips between DVE (13/16) and the Act
                # engine (3/16; Identity shares the loaded table with Exp).
                slot = t % OUTG
                if slot == 0:
                    epi_state["res"] = rp.tile([COUT, OUTG * SPB], bf16,
                                               tag="res", name=f"res_{b}_{t}")
                res = epi_state["res"]
                dst = res[:, slot * SPB:(slot + 1) * SPB]
                on_act = (t == 3) or (t == 6 and i % 2 == 0)
                if on_act:
                    nc.scalar.activation(dst, pt[:],
                                         mybir.ActivationFunctionType.Identity,
                                         bias=bv_t[:], scale=-1.0)
                else:
                    nc.vector.tensor_scalar(dst, pt[:], -1.0, bv_t[:],
                                            mybir.AluOpType.mult,
                                            mybir.AluOpType.add)
                if slot == OUTG - 1:
                    t0 = t - (OUTG - 1)
                    nc.sync.dma_start(
                        out_ap[b][:, t0 * ROWS_PER_TILE:(t + 1) * ROWS_PER_TILE, :],
                        res[:])

            def make_rhs(et, t, kw):
                # [p, 2(k-tile, stride PAD_TOT), 8 rows, 64 cols] — nested
                # non-overlapping strides (hw rejects overlapping k-tile APs)
                e4 = et.rearrange("p (r h w) -> p r h w", r=2, w=PAD_W)
                h0 = t * ROWS_PER_TILE
                return e4[0:128, :, h0:h0 + ROWS_PER_TILE, kw:kw + W]

            ets = {}
            for i, b in enumerate([b for _ in range(reps) for b in range(BL)]):
                xt = xp.tile([CIN, H * W], bf16, tag="xt", name=f"xt_{i}")
                nc.sync.dma_start(xt[:], x_ap[b].rearrange("c h w -> c (h w)"))
                if i == 0:
                    # after the first x DMA so x-data flows immediately
                    load_consts()
                xt3 = xt.rearrange("p (h w) -> p h w", w=W)
                bufslot = i % 2
                if bufslot not in ets:
                    # pad cells are written once per buffer and persist
                    # (later iterations only rewrite the interiors)
                    et = ep.tile([128, 2 * PAD_TOT], f8, tag="et",
                                 name=f"et_{bufslot}")
                    ets[bufslot] = et
                    e3 = et.rearrange("p (h w) -> p h w", w=PAD_W)
                    nc.gpsimd.memset(e3[0:64, 0, :], 1.0)            # top pad row
                    nc.gpsimd.memset(e3[0:64, H + 1, :], 1.0)        # bottom pad
                    nc.gpsimd.memset(e3[0:64, 1:H + 1, 0], 1.0)      # left col
                    nc.gpsimd.memset(e3[0:64, 1:H + 1, W + 1], 1.0)  # right col
                    # half1 row 65 + all of region1 half1 are read only
                    # against phantom zero weights: just keep them finite
                    nc.gpsimd.memset(e3[64:128, H + 1, :], 1.0)
                    nc.gpsimd.memset(et[64:128, PAD_TOT:PAD_TOT + H * PAD_W],
                                     1.0)
                et = ets[bufslot]
                et3 = et.rearrange("p (h w) -> p h w", w=PAD_W)
                # region0: half0 (partitions 0-63) = padded exp(x); half1 =
                # same shifted down one padded row. region1 (free offset
                # +PAD_TOT): half0 = E shifted down two rows.
                nc.scalar.activation(et3[0:CIN, 1:HH + 1, 1:W + 1],
                                     xt3[:, 0:HH, :], Exp)
                nc.scalar.activation(et3[0:CIN, HH + 1:H + 1, 1:W + 1],
                                     xt3[:, HH:H, :], Exp)
                # one merged dup (half1 = E+1row) and the region1 copy
                nc.sync.dma_start(et[64:128, 0:PAD_TOT - PAD_W],
                                  et[0:64, PAD_W:PAD_TOT])
                nc.sync.dma_start(et[0:64, PAD_TOT:PAD_TOT + H * PAD_W],
                                  et[0:64, 2 * PAD_W:PAD_TOT])

                pts = {t: pp.tile([COUT, ROWS_PER_TILE * W], f32,
                                  tag=f"pt{t}", bufs=1, name=f"pt_{i}_{t}")
                       for t in range(NTILES)}
                for g in range(3):
                    for t in range(NTILES):
                        nc.tensor.matmul(
                            pts[t][:], wdr4[:, g, :, :], make_rhs(et, t, g),
                            start=(g == 0), stop=(g == 2), perf_mode=DR)
                        if g == 2:
                            epilogue(i, b, t, pts[t])
    nc.compile()
    return nc


def _prep_weights(weight, bias):
    """wdr[p, g, k, o]: DoubleRow stationary layout, 3 groups x 2 k-tiles.
    partition p<64 -> half0 tap weight exp(w[o, p, tap0]); p>=64 -> half1
    tap (or 0 for phantom k-tiles)."""
    import ml_dtypes
    ew = np.exp(weight.astype(np.float32))           # [COUT, CIN, 3, 3]
    wdr = np.zeros((128, 3, 2, COUT), np.float32)
    for g in range(3):
        for k in range(2):
            tap0, tap1 = GROUP_TAPS[g][k]
            wdr[0:64, g, k, :] = ew[:, :, tap0[0], tap0[1]].T
            if tap1 is not None:
                wdr[64:128, g, k, :] = ew[:, :, tap1[0], tap1[1]].T
    wdr = wdr.reshape(128, 3 * 2 * COUT).astype(ml_dtypes.float8_e4m3)
    bvec = (KSUM + bias.astype(np.float32)).reshape(COUT, 1)
    return wdr, bvec


def _in_map(x_core, weight, bias):
    """Build the per-core input map. x_core: [BL, CIN, H, W] float32."""
    import ml_dtypes
    wdr, bvec = _prep_weights(weight, bias)
    x_bf = np.ascontiguousarray(x_core.astype(ml_dtypes.bfloat16))
    return {"x": x_bf, "wdr": wdr, "bvec": bvec}


def kernel(x, weight, bias):
    from concourse import bass_utils

    x = np.ascontiguousarray(np.asarray(x, dtype=np.float32))
    weight = np.asarray(weight, dtype=np.float32)
    bias = np.asarray(bias, dtype=np.float32)

    if "nc" not in _CACHE:
        _CACHE["nc"] = _build()
    nc = _CACHE["nc"]

    in_maps = [_in_map(x[c * BL:(c + 1) * BL], weight, bias)
               for c in range(NCORES)]
    res = bass_utils.run_bass_kernel_spmd(nc, in_maps, core_ids=list(range(NCORES)))
    return np.concatenate([np.asarray(r["out"], dtype=np.float32)
                           for r in res.results], axis=0)


# revision 13
# speedup vs baseline: 1.5086x; 1.0644x over previous
"""Trainium2 Bass kernel for nn_CustomConv2d_32538672234916.

out[b,o,h,w] = K - sum_{ci,kh,kw} exp(x_patch)*exp(w) + bias[o],  K = Cin*kh*kw = 576
i.e. out = (K + bias) - conv2d(exp(x) [1-padded], exp(weight), stride 1)

Sharding: data-parallel over batch B=16 across 8 cores (2 batches/core),
weights/bias replicated.

Per-core GEMM formulation (fp8 DoubleRow): SBUF holds exp(x) in fp8 three
times: region0 half0 (partitions 0-63) = padded exp(x) "E", region0 half1
(partitions 64-127) = E shifted down one padded row, region1 (free offset
+PAD_TOT) half0 = E shifted down two rows (region1 half1 is only ever
multiplied by phantom zero weights). Each 8-row output strip (512 px)
accumulates THREE MatmulPerfMode.DoubleRow fp8 matmuls (0.5 PE-cycles/row;
k-tile-dim stride = PAD_TOT, nested/non-overlapping -- the hw AP walker
rejects overlapping k-tile strides). Matmul g=kw covers the tap column:
  ktile0 -> taps (0,kw) [half0] + (1,kw) [half1]
  ktile1 -> taps (2,kw) [region1 half0] + phantom w=0 [region1 half1]
Tensor time ~= 16 strips * 3 * 256 cyc = 5.1us/core.

IO: x uploaded bf16 (1MB/core), exp runs bf16->fp8 on the Act engine, out
written bf16 (2MB/core) and widened to f32 on host. Epilogue (PSUM f32 *-1
+ (K+bias) -> bf16) alternates DVE / GpSimd so neither engine is the wall.
fp8 end-to-end rel err ~7e-3 vs the 2e-2 gate.
"""
import sys
sys.path.insert(0, '/opt/trn_rl_repo')
import numpy as np

B, CIN, H, W = 16, 64, 64, 64
COUT = 128
NCORES = 8
BL = B // NCORES          # batches per core
PAD_W = W + 2             # 66
PAD_TOT = PAD_W * (H + 2) # 66*66 = 4356
KSUM = float(CIN * 9)     # 576
ROWS_PER_TILE = 8
NTILES = H // ROWS_PER_TILE  # 8 strips per image
HH = H // 2

# taps covered: [(half0 tap, half1 tap), ...] per (group=kw, ktile);
# ktile0 reads region0 = (E, E+1row), ktile1 reads region1 = (E+2rows, any)
# at k-tile-dim stride PAD_TOT; None = phantom zero weight.
GROUP_TAPS = [
    [((0, 0), (1, 0)), ((2, 0), None)],
    [((0, 1), (1, 1)), ((2, 1), None)],
    [((0, 2), (1, 2)), ((2, 2), None)],
]

_CACHE = {}


def _build(reps=1):
    from concourse import bacc, mybir
    from concourse.tile import TileContext

    f32 = mybir.dt.float32
    bf16 = mybir.dt.bfloat16
    f8 = mybir.dt.float8e4
    Exp = mybir.ActivationFunctionType.Exp
    DR = mybir.MatmulPerfMode.DoubleRow

    nc = bacc.Bacc("TRN2", target_bir_lowering=False, debug=False)
    x_d = nc.dram_tensor("x", [BL, CIN, H, W], bf16, kind="ExternalInput")
    wdr_d = nc.dram_tensor("wdr", [128, 3 * 2 * COUT], f8, kind="ExternalInput")
    bvec_d = nc.dram_tensor("bvec", [COUT, 1], f32, kind="ExternalInput")
    out_d = nc.dram_tensor("out", [BL, COUT, H, W], bf16, kind="ExternalOutput")
    x_ap = x_d.ap()
    out_ap = out_d.ap()

    with TileContext(nc) as tc:
        with tc.tile_pool(name="consts", bufs=1) as consts, \
             tc.tile_pool(name="xp", bufs=2) as xp, \
             tc.tile_pool(name="ep", bufs=2) as ep, \
             tc.tile_pool(name="rp", bufs=4) as rp, \
             tc.tile_pool(name="pp", bufs=1, space="PSUM") as pp:
            wdr_t = consts.tile([128, 3 * 2 * COUT], f8)
            bv_t = consts.tile([COUT, 1], f32)
            wdr4 = wdr_t.rearrange("p (g k m) -> p g k m", g=3, k=2)

            def load_consts():
                nc.sync.dma_start(wdr_t[:], wdr_d.ap())
                nc.sync.dma_start(bv_t[:], bvec_d.ap())

            # out-DMA granularity: OUTG strips per transfer. Fewer DMA
            # instructions = less (shared) descriptor-gen serialization.
            OUTG = 4
            SPB = ROWS_PER_TILE * W
            epi_state = {}

            def epilogue(i, b, t, pt):
                # Evacuate a PAIR of strips (one PSUM tile spanning 2 banks)
                # per instruction: PSUM f32 * -1 + (K+bias) -> bf16. GPSIMD
                # can't read PSUM on this target, so pairs split between DVE
                # (7/8) and the Act engine (1/8; Identity shares the loaded
                # act table with Exp). Fires on the pair's second strip.
                if t % 2 == 0:
                    return
                slot = t % OUTG
                if slot == 1:
                    epi_state["res"] = rp.tile([COUT, OUTG * SPB], bf16,
                                               tag="res", name=f"res_{b}_{t}")
                res = epi_state["res"]
                dst = res[:, (slot - 1) * SPB:(slot + 1) * SPB]
                on_act = (t == 3 and i % 2 == 0)
                if on_act:
                    nc.scalar.activation(dst, pt[:],
                                         mybir.ActivationFunctionType.Identity,
                                         bias=bv_t[:], scale=-1.0)
                else:
                    nc.vector.tensor_scalar(dst, pt[:], -1.0, bv_t[:],
                                            mybir.AluOpType.mult,
                                            mybir.AluOpType.add)
                if slot == OUTG - 1:
                    t0 = t - (OUTG - 1)
                    nc.sync.dma_start(
                        out_ap[b][:, t0 * ROWS_PER_TILE:(t + 1) * ROWS_PER_TILE, :],
                        res[:])

            def make_rhs(et, t, kw):
                # [p, 2(k-tile, stride PAD_TOT), 8 rows, 64 cols] — nested
                # non-overlapping strides (hw rejects overlapping k-tile APs)
                e4 = et.rearrange("p (r h w) -> p r h w", r=2, w=PAD_W)
                h0 = t * ROWS_PER_TILE
                return e4[0:128, :, h0:h0 + ROWS_PER_TILE, kw:kw + W]

            ets = {}
            for i, b in enumerate([b for _ in range(reps) for b in range(BL)]):
                xt = xp.tile([CIN, H * W], bf16, tag="xt", name=f"xt_{i}")
                nc.sync.dma_start(xt[:], x_ap[b].rearrange("c h w -> c (h w)"))
                if i == 0:
                    # after the first x DMA so x-data flows immediately
                    load_consts()
                xt3 = xt.rearrange("p (h w) -> p h w", w=W)
                bufslot = i % 2
                if bufslot not in ets:
                    # pad cells are written once per buffer and persist
                    # (later iterations only rewrite the interiors)
                    et = ep.tile([128, 2 * PAD_TOT], f8, tag="et",
                                 name=f"et_{bufslot}")
                    ets[bufslot] = et
                    e3 = et.rearrange("p (h w) -> p h w", w=PAD_W)
                    nc.gpsimd.memset(e3[0:64, 0, :], 1.0)            # top pad row
                    nc.gpsimd.memset(e3[0:64, H + 1, :], 1.0)        # bottom pad
                    nc.gpsimd.memset(e3[0:64, 1:H + 1, 0], 1.0)      # left col
                    nc.gpsimd.memset(e3[0:64, 1:H + 1, W + 1], 1.0)  # right col
                    # half1 row 65 + all of region1 half1 are read only
                    # against phantom zero weights: just keep them finite
                    nc.gpsimd.memset(e3[64:128, H + 1, :], 1.0)
                    nc.gpsimd.memset(et[64:128, PAD_TOT:PAD_TOT + H * PAD_W],
                                     1.0)
                et = ets[bufslot]
                et3 = et.rearrange("p (h w) -> p h w", w=PAD_W)
                # region0: half0 (partitions 0-63) = padded exp(x); half1 =
                # same shifted down one padded row. region1 (free offset
                # +PAD_TOT): half0 = E shifted down two rows.
                # band-wise prep: each act half is chased immediately by its
                # dup (half1 = E+1row) and region1 (= E+2rows) copy bands so
                # early strips' matmuls unlock while the second half runs.
                nc.scalar.activation(et3[0:CIN, 1:HH + 1, 1:W + 1],
                                     xt3[:, 0:HH, :], Exp)
                nc.sync.dma_start(et[64:128, 0:HH * PAD_W],
                                  et[0:64, PAD_W:(HH + 1) * PAD_W])
                nc.sync.dma_start(et[0:64, PAD_TOT:PAD_TOT + (HH - 1) * PAD_W],
                                  et[0:64, 2 * PAD_W:(HH + 1) * PAD_W])
                nc.scalar.activation(et3[0:CIN, HH + 1:H + 1, 1:W + 1],
                                     xt3[:, HH:H, :], Exp)
                nc.sync.dma_start(et[64:128, HH * PAD_W:PAD_TOT - PAD_W],
                                  et[0:64, (HH + 1) * PAD_W:PAD_TOT])
                nc.sync.dma_start(
                    et[0:64, PAD_TOT + (HH - 1) * PAD_W:PAD_TOT + H * PAD_W],
                    et[0:64, (HH + 1) * PAD_W:PAD_TOT])

                # one PSUM tile per STRIP-PAIR (2 banks); each matmul still
                # targets a single bank-sized slice
                ptp = {p: pp.tile([COUT, 2 * SPB], f32,
                                  tag=f"pt{p}", bufs=1, name=f"pt_{i}_{p}")
                       for p in range(NTILES // 2)}
                for g in range(3):
                    for t in range(NTILES):
                        sl = ptp[t // 2][:, (t % 2) * SPB:(t % 2 + 1) * SPB]
                        nc.tensor.matmul(
                            sl, wdr4[:, g, :, :], make_rhs(et, t, g),
                            start=(g == 0), stop=(g == 2), perf_mode=DR)
                        if g == 2:
                            epilogue(i, b, t, ptp[t // 2])
    nc.compile()
    return nc


def _prep_weights(weight, bias):
    """wdr[p, g, k, o]: DoubleRow stationary layout, 3 groups x 2 k-tiles.
    partition p<64 -> half0 tap weight exp(w[o, p, tap0]); p>=64 -> half1
    tap (or 0 for phantom k-tiles)."""
    import ml_dtypes
    ew = np.exp(weight.astype(np.float32))           # [COUT, CIN, 3, 3]
    wdr = np.zeros((128, 3, 2, COUT), np.float32)
    for g in range(3):
        for k in range(2):
            tap0, tap1 = GROUP_TAPS[g][k]
            wdr[0:64, g, k, :] = ew[:, :, tap0[0], tap0[1]].T
            if tap1 is not None:
                wdr[64:128, g, k, :] = ew[:, :, tap1[0], tap1[1]].T
    wdr = wdr.reshape(128, 3 * 2 * COUT).astype(ml_dtypes.float8_e4m3)
    bvec = (KSUM + bias.astype(np.float32)).reshape(COUT, 1)
    return wdr, bvec


def _in_map(x_core, weight, bias):
    """Build the per-core input map. x_core: [BL, CIN, H, W] float32."""
    import ml_dtypes
    wdr, bvec = _prep_weights(weight, bias)
    x_bf = np.ascontiguousarray(x_core.astype(ml_dtypes.bfloat16))
    return {"x": x_bf, "wdr": wdr, "bvec": bvec}


def kernel(x, weight, bias):
    from concourse import bass_utils

    x = np.ascontiguousarray(np.asarray(x, dtype=np.float32))
    weight = np.asarray(weight, dtype=np.float32)
    bias = np.asarray(bias, dtype=np.float32)

    if "nc" not in _CACHE:
        _CACHE["nc"] = _build()
    nc = _CACHE["nc"]

    in_maps = [_in_map(x[c * BL:(c + 1) * BL], weight, bias)
               for c in range(NCORES)]
    res = bass_utils.run_bass_kernel_spmd(nc, in_maps, core_ids=list(range(NCORES)))
    return np.concatenate([np.asarray(r["out"], dtype=np.float32)
                           for r in res.results], axis=0)
